# revision 9
# baseline (speedup 1.0000x reference)
"""CQT (constant-Q transform) kernel for Trainium2, 8 NeuronCores.

Math: out[b, c, t] = sum_l W[c, l] * x_pad[b, t*HOP + l]   (strided conv,
HOP=512, L=11339 taps, C=168 channels = 84 bins x re/im), then reshaped to
(B, 2, n_bins, T_out).

Strategy:
  - Data-parallel: shard B=32 across 8 cores (4 batches/core), weights
    replicated.
  - The conv is decomposed into 128-tap blocks: block p covers taps
    [128p, 128p+128).  The moving operand for block p=(4j+k) at output
    tile [t0, t0+nt) is a contiguous column slice of a host-pre-transposed
    view of x:  xt[r, k, u] = x_pad[512u + 128k + r].
  - CQT kernels are ragged (bin k has ~11339*2^(-k/12) taps, centered), so
    most blocks touch only a few low-bin channels.  A plain matmul costs
    ~N streaming cycles regardless of how few of the 128 PE columns hold
    weights, so the dense-block formulation wastes most of the array.
  - Column tiling: channels are split into groups of 32 (16 bins).  Each
    (block, group) quantum is a K=128, M<=32, N=nt matmul placed on one of
    the four 32-column PE tile positions (tile_position=(0, 32*slot)).
    The 4 tile positions stream concurrently, quartering PE time.
    Quanta per t-tile per group: {89, 36, 15, 7, 3, 2} = 152 vs 92
    full-width matmuls for the dense-block formulation; packed on 4 slots
    the makespan is 114 passes/batch vs 276 -> ~2.4x less PE streaming.
  - Each (group, t-tile) job accumulates its blocks into its own PSUM bank
    (per-element has_written semantics: first write overwrites, later ones
    accumulate), then DVE-copies psum[32s:32s+m] -> SBUF and DMAs to out.
    Static balanced schedule: 4 slots x 114 passes per batch.
"""

import numpy as np

HOP = 512
N_CORES = 8

_prog_cache: dict = {}


def _host_prep(x, kernels):
    x = np.ascontiguousarray(np.asarray(x, dtype=np.float32))
    kernels = np.ascontiguousarray(np.asarray(kernels, dtype=np.float32))
    B, T = x.shape
    nbins, two, Lmax = kernels.shape
    assert two == 2
    C = 2 * nbins
    pad = Lmax // 2
    T_out = (T + 2 * pad - Lmax) // HOP + 1

    # ---- weights: pad taps to 128 multiple ----
    nblk = -(-Lmax // 128)
    Wp = np.zeros((C, nblk * 128), dtype=np.float32)
    Wp[:, :Lmax] = kernels.reshape(C, Lmax)
    nzb = (Wp.reshape(C, nblk, 128) != 0.0).any(axis=2)  # [C, nblk]

    # channel groups of 32 (16 bins); bins are sorted by descending filter
    # length, supports are nested, so a group's active blocks = union over
    # its channels = the blocks of its longest (first) channel.
    groups = []  # (c0, m, blocks)
    for c0 in range(0, C, 32):
        m = min(32, C - c0)
        blks = np.where(nzb[c0:c0 + m].any(axis=0))[0].tolist()
        groups.append((c0, m, blks))

    # Weight layout: per (group, block) a zero-padded [128 taps, m chans]
    # panel; panels of a group are consecutive in block order.
    wcols = []
    tot = 0
    for (c0, m, blks) in groups:
        wcols.append(tot)
        tot += m * len(blks)
    wt = np.zeros((128, tot), dtype=np.float32)
    for (c0, m, blks), w0 in zip(groups, wcols):
        for rel, p in enumerate(blks):
            wt[:, w0 + rel * m: w0 + (rel + 1) * m] = \
                Wp[c0:c0 + m, 128 * p: 128 * (p + 1)].T
    import ml_dtypes
    wt = np.ascontiguousarray(wt.astype(ml_dtypes.bfloat16))

    # ---- x: pad and pre-transpose to [128, 4, U] per batch ----
    j_max = (nblk - 1) // 4
    U = T_out + j_max
    xpad_len = 512 * U
    assert xpad_len >= pad + T, (xpad_len, pad + T)
    xp = np.zeros((B, xpad_len), dtype=np.float32)
    xp[:, pad:pad + T] = x
    # xt[b, r, k*U + u] = xp[b, 512u + 128k + r]
    import ml_dtypes
    xt = np.ascontiguousarray(
        xp.reshape(B, U, 4, 128).transpose(0, 3, 2, 1).reshape(B, 128, 4 * U)
        .astype(ml_dtypes.bfloat16)
    )
    return xt, wt, groups, wcols, C, U, T_out, nbins


def _build_schedule(groups, T_out):
    """Static balanced 4-slot schedule.  Jobs are (group g, t-tile tt);
    job (g, tt) = len(groups[g].blocks) passes.  Slot loads for the CQT
    shape: 114/114/114/114 per batch."""
    nts = []
    t0 = 0
    while t0 < T_out:
        nts.append((t0, min(512, T_out - t0)))
        t0 += 512
    assert len(nts) == 3 and len(groups) == 6
    SLOT_JOBS = [
        [(0, 0), (2, 2), (3, 2), (4, 2)],
        [(1, 0), (1, 1), (1, 2), (5, 0), (5, 1), (5, 2)],
        [(2, 0), (3, 0), (4, 0), (0, 1)],
        [(2, 1), (3, 1), (4, 1), (0, 2)],
    ]
    # coverage check
    seen = set()
    for sj in SLOT_JOBS:
        for g, tt in sj:
            assert (g, tt) not in seen
            seen.add((g, tt))
    assert seen == {(g, tt) for g in range(6) for tt in range(3)}

    # flatten to per-slot quanta
    slot_q = []
    for sj in SLOT_JOBS:
        qs = []
        for g, tt in sj:
            blks = groups[g][2]
            for rel, p in enumerate(blks):
                qs.append((g, tt, p, rel, rel == 0, rel == len(blks) - 1))
        slot_q.append(qs)
    return nts, slot_q


def _build_program(b_per, C, U, T_out, groups, wcols):
    import concourse.mybir as mybir
    import concourse.tile as tile
    from concourse import bacc

    f32 = mybir.dt.float32
    bf16 = mybir.dt.bfloat16
    wtot = wcols[-1] + groups[-1][1] * len(groups[-1][2])
    nts, slot_q = _build_schedule(groups, T_out)
    maxlen = max(len(q) for q in slot_q)

    nc = bacc.Bacc(
        "TRN2",
        target_bir_lowering=False,
        debug=False,
        enable_asserts=True,
        num_devices=N_CORES,
    )
    xt_d = nc.dram_tensor("xt", [b_per, 128, 4 * U], bf16, kind="ExternalInput").ap()
    wt_d = nc.dram_tensor("wt", [128, wtot], bf16, kind="ExternalInput").ap()
    out_d = nc.dram_tensor("out", [b_per, C, T_out], f32, kind="ExternalOutput").ap()

    # weight DMA chunks, in consumption order: every slot consumes its
    # group's panels linearly from pass 0, so send a tiny head of each
    # group first, then medium heads, then the long tails.
    w_chunks = []  # (g, rel0, rel1)
    H1, H2 = 4, 20
    for g, (c0, m, blks) in enumerate(groups):
        w_chunks.append((g, 0, min(H1, len(blks))))
    for g, (c0, m, blks) in enumerate(groups):
        if len(blks) > H1:
            w_chunks.append((g, H1, min(H2, len(blks))))
    for g, (c0, m, blks) in enumerate(groups):
        if len(blks) > H2:
            w_chunks.append((g, H2, len(blks)))

    # x DMA chunk boundaries for batch 0 (u-ranges per t-tile window)
    j_max = (max(groups[0][2])) // 4
    x_stops = []
    for (t0_, nt_) in nts:
        x_stops.append(min(t0_ + nt_ + j_max + 1, U))
    x_stops[-1] = U

    with tile.TileContext(nc) as tc:
        with (
            tc.tile_pool(name="wpool", bufs=1) as wpool,
            tc.tile_pool(name="xpool", bufs=4) as xpool,
            tc.tile_pool(name="evpool", bufs=2) as evpool,
            tc.tile_pool(name="pspool", bufs=2, space="PSUM") as pspool,
        ):
            wsb = wpool.tile([128, wtot], bf16)
            xbs = [
                xpool.tile([128, 4 * U], bf16, tag="xb", name=f"xb{b}")
                for b in range(b_per)
            ]

            # weights on the scalar engine's DMA queue, in consumption order
            for g, r0, r1 in w_chunks:
                c0, m, blks = groups[g]
                a0 = wcols[g] + r0 * m
                a1 = wcols[g] + r1 * m
                nc.scalar.dma_start(out=wsb[:, a0:a1], in_=wt_d[:, a0:a1])

            # batch-0 x split across the vector and tensor engines' DMA
            # queues (parallel to the weight stream); later batches
            # prefetched whole, spread over sync/vector/tensor queues.
            src0 = xt_d[0].rearrange("r (k u) -> r k u", k=4)
            dst0 = xbs[0].rearrange("r (k u) -> r k u", k=4)
            u1 = x_stops[0]
            nc.gpsimd.dma_start(out=dst0[:, 0:2, 0:u1], in_=src0[:, 0:2, 0:u1])
            nc.sync.dma_start(out=dst0[:, 2:4, 0:u1], in_=src0[:, 2:4, 0:u1])
            nc.gpsimd.dma_start(
                out=dst0[:, :, u1:x_stops[1]], in_=src0[:, :, u1:x_stops[1]]
            )
            nc.sync.dma_start(
                out=dst0[:, :, x_stops[1]:U], in_=src0[:, :, x_stops[1]:U]
            )
            qs = [nc.gpsimd, nc.sync, nc.gpsimd]
            for b in range(1, b_per):
                qs[(b - 1) % 3].dma_start(out=xbs[b][:], in_=xt_d[b])

            for b in range(b_per):
                xb = xbs[b]

                cur_ps = [None] * 4
                for i in range(maxlen):
                    for s in range(4):
                        if i >= len(slot_q[s]):
                            continue
                        g, tt, p, rel, first, last = slot_q[s][i]
                        c0, m, blks = groups[g]
                        t0, nt = nts[tt]
                        if first:
                            cur_ps[s] = pspool.tile(
                                [128, 512], f32, tag=f"ps{s}", name=f"ps{s}_{b}_{g}_{tt}"
                            )
                        ps = cur_ps[s]
                        j, k = divmod(p, 4)
                        wc = wcols[g] + rel * m
                        nc.tensor.matmul(
                            ps[32 * s: 32 * s + m, :nt],
                            lhsT=wsb[:, wc: wc + m],
                            rhs=xb[:, k * U + t0 + j: k * U + t0 + j + nt],
                            start=first,
                            stop=last,
                            tile_position=(0, 32 * s),
                        )
                        if last:
                            ev = evpool.tile(
                                [128, 512], f32, tag=f"ev{s}", name=f"ev{s}_{b}_{g}_{tt}"
                            )
                            nc.vector.tensor_copy(
                                ev[32 * s: 32 * s + m, :nt],
                                ps[32 * s: 32 * s + m, :nt],
                            )
                            nc.sync.dma_start(
                                out=out_d[b, c0:c0 + m, t0:t0 + nt],
                                in_=ev[32 * s: 32 * s + m, :nt],
                            )
    nc.compile()
    return nc


def _ensure_trace_shims():
    """If run_bass_kernel_spmd is invoked with tracing enabled (e.g. via
    BASS_TRACE=1) it imports antenv.axon_hooks and uploads artifacts to a
    bucket; neither exists in a bare container.  Register a working NTFF
    hook (ctypes into the axon .so) and a no-op uploader so the trace path
    degrades gracefully instead of crashing."""
    import sys

    try:
        import antenv.axon_hooks  # noqa: F401
    except ImportError:
        import contextlib
        import ctypes
        import types

        hook = None
        try:
            lib = ctypes.CDLL("/opt/axon/libaxon_pjrt.so")
            if hasattr(lib, "axon_start_nrt_profile"):
                lib.axon_start_nrt_profile.argtypes = [
                    ctypes.POINTER(ctypes.c_int64),
                    ctypes.c_size_t,
                ]
                lib.axon_start_nrt_profile.restype = ctypes.c_int64
                lib.axon_stop_nrt_profile.argtypes = [ctypes.c_char_p]
                lib.axon_stop_nrt_profile.restype = ctypes.c_int64

                @contextlib.contextmanager
                def _hook(output_dir, device_ids):
                    import jax

                    jax.devices()
                    if device_ids:
                        ids = (ctypes.c_int64 * len(device_ids))(*device_ids)
                        rc = lib.axon_start_nrt_profile(ids, len(device_ids))
                    else:
                        rc = lib.axon_start_nrt_profile(None, 0)
                    if rc != 0:
                        raise RuntimeError(f"axon_start_nrt_profile rc={rc}")
                    try:
                        yield
                    finally:
                        lib.axon_stop_nrt_profile(str(output_dir).encode())

                hook = _hook
        except OSError:
            pass
        mod = types.ModuleType("antenv.axon_hooks")
        mod.get_axon_ntff_profile_hook = lambda: hook
        mod.set_axon_ntff_profile_hook = lambda h: None
        sys.modules["antenv.axon_hooks"] = mod

    try:
        import concourse.bass_utils as _bu

        _orig_upload = _bu.upload_artifacts

        def _safe_upload(tmpdir):
            try:
                return _orig_upload(tmpdir)
            except Exception:
                return "local://unavailable"

        if not getattr(_bu, "_safe_upload_installed", False):
            _bu.upload_artifacts = _safe_upload
            _bu._safe_upload_installed = True
    except Exception:
        pass


def kernel(x, kernels):
    _ensure_trace_shims()
    from concourse.bass_utils import run_bass_kernel_spmd

    xt, wt, groups, wcols, C, U, T_out, nbins = _host_prep(x, kernels)
    B = xt.shape[0]
    assert B % N_CORES == 0
    b_per = B // N_CORES

    key = (b_per, C, U, T_out, tuple((c0, m, tuple(b)) for c0, m, b in groups))
    if key not in _prog_cache:
        _prog_cache[key] = _build_program(b_per, C, U, T_out, groups, wcols)
    nc = _prog_cache[key]

    in_maps = [
        {"xt": xt[c * b_per:(c + 1) * b_per], "wt": wt} for c in range(N_CORES)
    ]
    res = run_bass_kernel_spmd(nc, in_maps, list(range(N_CORES)))
    parts = [res.results[c]["out"] for c in range(N_CORES)]
    out = np.concatenate(parts, axis=0)  # (B, C, T_out)
    return np.ascontiguousarray(
        out.reshape(B, nbins, 2, T_out).transpose(0, 2, 1, 3)
    )


# revision 12
# speedup vs baseline: 1.0543x; 1.0543x over previous
"""CQT (constant-Q transform) kernel for Trainium2, 8 NeuronCores.

Math: out[b, c, t] = sum_l W[c, l] * x_pad[b, t*HOP + l]   (strided conv,
HOP=512, L=11339 taps, C=168 channels = 84 bins x re/im), then reshaped to
(B, 2, n_bins, T_out).

Strategy:
  - Data-parallel: shard B=32 across 8 cores (4 batches/core), weights
    replicated.
  - The conv is decomposed into 128-tap blocks: block p covers taps
    [128p, 128p+128).  The moving operand for block p=(4j+k) at output
    tile [t0, t0+nt) is a contiguous column slice of a host-pre-transposed
    view of x:  xt[r, k, u] = x_pad[512u + 128k + r].
  - CQT kernels are ragged (bin k has ~11339*2^(-k/12) taps, centered), so
    most blocks touch only a few low-bin channels.  A plain matmul costs
    ~N streaming cycles regardless of how few of the 128 PE columns hold
    weights, so the dense-block formulation wastes most of the array.
  - Column tiling: channels are split into groups of 32 (16 bins).  Each
    (block, group) quantum is a K=128, M<=32, N=nt matmul placed on one of
    the four 32-column PE tile positions (tile_position=(0, 32*slot)).
    The 4 tile positions stream concurrently, quartering PE time.
    Quanta per t-tile per group: {89, 36, 15, 7, 3, 2} = 152 vs 92
    full-width matmuls for the dense-block formulation; packed on 4 slots
    the makespan is 114 passes/batch vs 276 -> ~2.4x less PE streaming.
  - Each (group, t-tile) job accumulates its blocks into its own PSUM bank
    (per-element has_written semantics: first write overwrites, later ones
    accumulate), then DVE-copies psum[32s:32s+m] -> SBUF and DMAs to out.
    Static balanced schedule: 4 slots x 114 passes per batch.
"""

import numpy as np

HOP = 512
N_CORES = 8

_prog_cache: dict = {}


def _host_prep(x, kernels):
    x = np.ascontiguousarray(np.asarray(x, dtype=np.float32))
    kernels = np.ascontiguousarray(np.asarray(kernels, dtype=np.float32))
    B, T = x.shape
    nbins, two, Lmax = kernels.shape
    assert two == 2
    C = 2 * nbins
    pad = Lmax // 2
    T_out = (T + 2 * pad - Lmax) // HOP + 1

    # ---- weights: pad taps to 128 multiple ----
    nblk = -(-Lmax // 128)
    Wp = np.zeros((C, nblk * 128), dtype=np.float32)
    Wp[:, :Lmax] = kernels.reshape(C, Lmax)
    nzb = (Wp.reshape(C, nblk, 128) != 0.0).any(axis=2)  # [C, nblk]

    # channel groups of 32 (16 bins); bins are sorted by descending filter
    # length, supports are nested, so a group's active blocks = union over
    # its channels = the blocks of its longest (first) channel.
    groups = []  # (c0, m, blocks)
    for c0 in range(0, C, 32):
        m = min(32, C - c0)
        blks = np.where(nzb[c0:c0 + m].any(axis=0))[0].tolist()
        groups.append((c0, m, blks))

    # Weight layout: per (group, block) a zero-padded [128 taps, m chans]
    # panel; panels of a group are consecutive in block order.
    wcols = []
    tot = 0
    for (c0, m, blks) in groups:
        wcols.append(tot)
        tot += m * len(blks)
    wt = np.zeros((128, tot), dtype=np.float32)
    for (c0, m, blks), w0 in zip(groups, wcols):
        for rel, p in enumerate(blks):
            wt[:, w0 + rel * m: w0 + (rel + 1) * m] = \
                Wp[c0:c0 + m, 128 * p: 128 * (p + 1)].T
    import ml_dtypes
    wt = np.ascontiguousarray(wt.astype(ml_dtypes.bfloat16))

    # ---- x: pad and pre-transpose to [128, 4, U] per batch ----
    j_max = (nblk - 1) // 4
    U = T_out + j_max
    xpad_len = 512 * U
    assert xpad_len >= pad + T, (xpad_len, pad + T)
    xp = np.zeros((B, xpad_len), dtype=np.float32)
    xp[:, pad:pad + T] = x
    # xt[b, r, k*U + u] = xp[b, 512u + 128k + r]
    import ml_dtypes
    xt = np.ascontiguousarray(
        xp.reshape(B, U, 4, 128).transpose(0, 3, 2, 1).reshape(B, 128, 4 * U)
        .astype(ml_dtypes.bfloat16)
    )
    return xt, wt, groups, wcols, C, U, T_out, nbins


def _build_schedule(groups, T_out):
    """Static balanced 4-slot schedule.  Jobs are (group g, t-tile tt);
    job (g, tt) = len(groups[g].blocks) passes.  Slot loads for the CQT
    shape: 114/114/114/114 per batch."""
    nts = []
    t0 = 0
    while t0 < T_out:
        nts.append((t0, min(512, T_out - t0)))
        t0 += 512
    assert len(nts) == 3 and len(groups) == 6
    SLOT_JOBS = [
        [(0, 0), (2, 2), (3, 2), (4, 2)],
        [(1, 0), (1, 1), (1, 2), (5, 0), (5, 1), (5, 2)],
        [(2, 0), (3, 0), (4, 0), (0, 1)],
        [(2, 1), (3, 1), (4, 1), (0, 2)],
    ]
    # coverage check
    seen = set()
    for sj in SLOT_JOBS:
        for g, tt in sj:
            assert (g, tt) not in seen
            seen.add((g, tt))
    assert seen == {(g, tt) for g in range(6) for tt in range(3)}

    # flatten to per-slot quanta
    slot_q = []
    for sj in SLOT_JOBS:
        qs = []
        for g, tt in sj:
            blks = groups[g][2]
            for rel, p in enumerate(blks):
                qs.append((g, tt, p, rel, rel == 0, rel == len(blks) - 1))
        slot_q.append(qs)
    return nts, slot_q


def _build_program(b_per, C, U, T_out, groups, wcols):
    import concourse.mybir as mybir
    import concourse.tile as tile
    from concourse import bacc

    f32 = mybir.dt.float32
    bf16 = mybir.dt.bfloat16
    wtot = wcols[-1] + groups[-1][1] * len(groups[-1][2])
    nts, slot_q = _build_schedule(groups, T_out)
    maxlen = max(len(q) for q in slot_q)

    nc = bacc.Bacc(
        "TRN2",
        target_bir_lowering=False,
        debug=False,
        enable_asserts=True,
        num_devices=N_CORES,
    )
    xt_d = nc.dram_tensor("xt", [b_per, 128, 4 * U], bf16, kind="ExternalInput").ap()
    wt_d = nc.dram_tensor("wt", [128, wtot], bf16, kind="ExternalInput").ap()
    out_d = nc.dram_tensor("out", [b_per, C, T_out], f32, kind="ExternalOutput").ap()

    # weight DMA chunks, in consumption order: every slot consumes its
    # group's panels linearly from pass 0, so send a tiny head of each
    # group first, then medium heads, then the long tails.
    w_head, w_rest = [], []  # (g, rel0, rel1)
    H1, H2 = 6, 24
    for g, (c0, m, blks) in enumerate(groups):
        w_head.append((g, 0, min(H1, len(blks))))
    for g, (c0, m, blks) in enumerate(groups):
        if len(blks) > H1:
            w_rest.append((g, H1, min(H2, len(blks))))
    for g, (c0, m, blks) in enumerate(groups):
        if len(blks) > H2:
            w_rest.append((g, H2, len(blks)))

    # x DMA chunk boundaries for batch 0 (u-ranges per t-tile window)
    j_max = (max(groups[0][2])) // 4
    x_stops = []
    for (t0_, nt_) in nts:
        x_stops.append(min(t0_ + nt_ + j_max + 1, U))
    x_stops[-1] = U

    with tile.TileContext(nc) as tc:
        with (
            tc.tile_pool(name="wpool", bufs=1) as wpool,
            tc.tile_pool(name="xpool", bufs=4) as xpool,
            tc.tile_pool(name="evpool", bufs=2) as evpool,
            tc.tile_pool(name="pspool", bufs=2, space="PSUM") as pspool,
        ):
            wsb = wpool.tile([128, wtot], bf16)
            xbs = [
                xpool.tile([128, 4 * U], bf16, tag="xb", name=f"xb{b}")
                for b in range(b_per)
            ]

            # Two HW DMA queues (sync=SP, scalar=Activation).  Interleave
            # the pass-0 critical set — weight heads + x windows for all
            # three t-tiles — across both, in consumption order.
            def w_dma(eng, ch):
                g, r0, r1 = ch
                c0, m, blks = groups[g]
                a0 = wcols[g] + r0 * m
                a1 = wcols[g] + r1 * m
                eng.dma_start(out=wsb[:, a0:a1], in_=wt_d[:, a0:a1])

            src0 = xt_d[0].rearrange("r (k u) -> r k u", k=4)
            dst0 = xbs[0].rearrange("r (k u) -> r k u", k=4)
            u1 = x_stops[0]
            u2 = x_stops[1]
            # scalar: w heads, x k-planes 2-3, then w rest
            for ch in w_head:
                w_dma(nc.scalar, ch)
            nc.scalar.dma_start(out=dst0[:, 2:4, 0:u1], in_=src0[:, 2:4, 0:u1])
            nc.scalar.dma_start(out=dst0[:, 2:4, u1:u2], in_=src0[:, 2:4, u1:u2])
            for ch in w_rest:
                w_dma(nc.scalar, ch)
            # sync: x k-planes 0-1, x tail window, then whole-batch prefetch
            nc.sync.dma_start(out=dst0[:, 0:2, 0:u1], in_=src0[:, 0:2, 0:u1])
            nc.sync.dma_start(out=dst0[:, 0:2, u1:u2], in_=src0[:, 0:2, u1:u2])
            nc.sync.dma_start(out=dst0[:, :, u2:U], in_=src0[:, :, u2:U])
            for b in range(1, b_per):
                nc.sync.dma_start(out=xbs[b][:], in_=xt_d[b])

            for b in range(b_per):
                xb = xbs[b]

                cur_ps = [None] * 4
                for i in range(maxlen):
                    for s in range(4):
                        if i >= len(slot_q[s]):
                            continue
                        g, tt, p, rel, first, last = slot_q[s][i]
                        c0, m, blks = groups[g]
                        t0, nt = nts[tt]
                        if first:
                            cur_ps[s] = pspool.tile(
                                [128, 512], f32, tag=f"ps{s}", name=f"ps{s}_{b}_{g}_{tt}"
                            )
                        ps = cur_ps[s]
                        j, k = divmod(p, 4)
                        wc = wcols[g] + rel * m
                        nc.tensor.matmul(
                            ps[32 * s: 32 * s + m, :nt],
                            lhsT=wsb[:, wc: wc + m],
                            rhs=xb[:, k * U + t0 + j: k * U + t0 + j + nt],
                            start=first,
                            stop=last,
                            tile_position=(0, 32 * s),
                        )
                        if last:
                            ev = evpool.tile(
                                [128, 512], f32, tag=f"ev{s}", name=f"ev{s}_{b}_{g}_{tt}"
                            )
                            if s < 2:
                                nc.vector.tensor_copy(
                                    ev[32 * s: 32 * s + m, :nt],
                                    ps[32 * s: 32 * s + m, :nt],
                                )
                                nc.sync.dma_start(
                                    out=out_d[b, c0:c0 + m, t0:t0 + nt],
                                    in_=ev[32 * s: 32 * s + m, :nt],
                                )
                            else:
                                nc.scalar.copy(
                                    ev[32 * s: 32 * s + m, :nt],
                                    ps[32 * s: 32 * s + m, :nt],
                                )
                                nc.scalar.dma_start(
                                    out=out_d[b, c0:c0 + m, t0:t0 + nt],
                                    in_=ev[32 * s: 32 * s + m, :nt],
                                )
    nc.compile()
    return nc


def _ensure_trace_shims():
    """If run_bass_kernel_spmd is invoked with tracing enabled (e.g. via
    BASS_TRACE=1) it imports antenv.axon_hooks and uploads artifacts to a
    bucket; neither exists in a bare container.  Register a working NTFF
    hook (ctypes into the axon .so) and a no-op uploader so the trace path
    degrades gracefully instead of crashing."""
    import sys

    try:
        import antenv.axon_hooks  # noqa: F401
    except ImportError:
        import contextlib
        import ctypes
        import types

        hook = None
        try:
            lib = ctypes.CDLL("/opt/axon/libaxon_pjrt.so")
            if hasattr(lib, "axon_start_nrt_profile"):
                lib.axon_start_nrt_profile.argtypes = [
                    ctypes.POINTER(ctypes.c_int64),
                    ctypes.c_size_t,
                ]
                lib.axon_start_nrt_profile.restype = ctypes.c_int64
                lib.axon_stop_nrt_profile.argtypes = [ctypes.c_char_p]
                lib.axon_stop_nrt_profile.restype = ctypes.c_int64

                @contextlib.contextmanager
                def _hook(output_dir, device_ids):
                    import jax

                    jax.devices()
                    if device_ids:
                        ids = (ctypes.c_int64 * len(device_ids))(*device_ids)
                        rc = lib.axon_start_nrt_profile(ids, len(device_ids))
                    else:
                        rc = lib.axon_start_nrt_profile(None, 0)
                    if rc != 0:
                        raise RuntimeError(f"axon_start_nrt_profile rc={rc}")
                    try:
                        yield
                    finally:
                        lib.axon_stop_nrt_profile(str(output_dir).encode())

                hook = _hook
        except OSError:
            pass
        mod = types.ModuleType("antenv.axon_hooks")
        mod.get_axon_ntff_profile_hook = lambda: hook
        mod.set_axon_ntff_profile_hook = lambda h: None
        sys.modules["antenv.axon_hooks"] = mod

    try:
        import concourse.bass_utils as _bu

        _orig_upload = _bu.upload_artifacts

        def _safe_upload(tmpdir):
            try:
                return _orig_upload(tmpdir)
            except Exception:
                return "local://unavailable"

        if not getattr(_bu, "_safe_upload_installed", False):
            _bu.upload_artifacts = _safe_upload
            _bu._safe_upload_installed = True
    except Exception:
        pass


def kernel(x, kernels):
    _ensure_trace_shims()
    from concourse.bass_utils import run_bass_kernel_spmd

    xt, wt, groups, wcols, C, U, T_out, nbins = _host_prep(x, kernels)
    B = xt.shape[0]
    assert B % N_CORES == 0
    b_per = B // N_CORES

    key = (b_per, C, U, T_out, tuple((c0, m, tuple(b)) for c0, m, b in groups))
    if key not in _prog_cache:
        _prog_cache[key] = _build_program(b_per, C, U, T_out, groups, wcols)
    nc = _prog_cache[key]

    in_maps = [
        {"xt": xt[c * b_per:(c + 1) * b_per], "wt": wt} for c in range(N_CORES)
    ]
    res = run_bass_kernel_spmd(nc, in_maps, list(range(N_CORES)))
    parts = [res.results[c]["out"] for c in range(N_CORES)]
    out = np.concatenate(parts, axis=0)  # (B, C, T_out)
    return np.ascontiguousarray(
        out.reshape(B, nbins, 2, T_out).transpose(0, 2, 1, 3)
    )
